# revision 31
# baseline (speedup 1.0000x reference)
"""LIF spike kernel for Trainium2 (Bass/Tile), data-parallel over batch on 8 cores.

Reparametrized recurrence: with v_t = u_t * 2^t and host-prescaled
x'_t = x_t * 2^t (exact power-of-2 scaling), the LIF step needs no tau
multiply:
  v_t = m'_{t-1} + x'_t ; s_t = v_t > 2^t ; m'_t = (v_t <= 2^t) * v_t

Engine findings baked in: DVE+GpSimd compute thrash each other's SBUF
access (GpSimd runs nothing), DVE+Act coexist cleanly, fp32 DVE ops are
element-rate-bound. Batches fused in pairs -> [128, 2048] ops (half the
instruction overhead). m' scratch lives in PSUM (separate memory, so DVE
traffic does not contend with DMA writes into SBUF). Per (pair, t):
  s   = Act sign(v - 2^t) -> i8 tile -> out-DMA   (off the critical chain)
  m'  = stt(v, 2^t, v, is_le, mult)   DVE -> PSUM
  v'  = tt(m', x'_{t+1}, add)         DVE -> SBUF
Host layout per core: x_core [C=128, B_loc=4, T*HW=8192] f32 (prescaled);
output i8 [C, B_loc, T*HW], spike decoded as (raw == 1).
"""

import numpy as np

import concourse.bacc as bacc
import concourse.mybir as mybir
from concourse.tile import TileContext
from concourse.bass_utils import run_bass_kernel_spmd

B, T, C, H, W = 32, 8, 128, 32, 32
HW = H * W
N_CORES = 8
B_LOC = B // N_CORES
N_PAIR = B_LOC // 2

f32 = mybir.dt.float32
i8 = mybir.dt.int8
op = mybir.AluOpType
AF = mybir.ActivationFunctionType

_nc_cache = None


def build_nc():
    nc = bacc.Bacc("TRN2", target_bir_lowering=False)
    x = nc.dram_tensor("x", [C, B_LOC, T * HW], f32, kind="ExternalInput")
    out = nc.dram_tensor("out", [C, B_LOC, T * HW], i8, kind="ExternalOutput")

    def xsl(p, t):
        return x[:, 2 * p : 2 * p + 2, t * HW : (t + 1) * HW]

    with TileContext(nc) as tc:
        with (
            tc.tile_pool(name="xq", bufs=4) as xq,
            tc.tile_pool(name="vp", bufs=3) as vp,
            tc.tile_pool(name="pp", bufs=1, space="PSUM") as pp,
            tc.tile_pool(name="sp_", bufs=4) as spool,
            tc.tile_pool(name="cst", bufs=1) as cst,
        ):
            bias = []
            for t in range(T):
                bt = cst.tile([C, 1], f32, name=f"bias{t}")
                nc.vector.memset(bt[:], -float(2**t))
                bias.append(bt)

            warm = cst.tile([C, 1], i8, name="warm")
            nc.scalar.activation(warm[:], bias[0][:], AF.Sign, bias=bias[0][:])

            v_cur = [None] * N_PAIR
            xt_tiles = [[None] * T for _ in range(N_PAIR)]

            def issue_one(p, t):
                xt = xq.tile([C, 2 * HW], f32, tag=f"x{p}", name=f"x_{p}_{t}")
                nc.sync.dma_start(out=xt[:], in_=xsl(p, t))
                xt_tiles[p][t] = xt

            for p in range(N_PAIR):
                vt = vp.tile([C, 2 * HW], f32, tag=f"v{p}", name=f"v0_{p}")
                nc.sync.dma_start(out=vt[:], in_=xsl(p, 0))
                v_cur[p] = vt
                issue_one(p, 1)

            def issue_in(t):
                if 1 <= t < T:
                    for p in range(N_PAIR):
                        issue_one(p, t)

            issue_in(2)

            for t in range(T):
                issue_in(t + 3)
                thr = float(2**t)
                for p in range(N_PAIR):
                    v = v_cur[p]
                    st = spool.tile([C, 2 * HW], i8, tag=f"s{p}", name=f"s_{p}_{t}")
                    nc.scalar.activation(
                        st[:], v[:], AF.Sign, bias=bias[t][:], scale=1.0
                    )
                    nc.sync.dma_start(
                        out=out[:, 2 * p : 2 * p + 2, t * HW : (t + 1) * HW],
                        in_=st[:],
                    )
                    if t == T - 1:
                        continue
                    mt = pp.tile([C, 2 * HW], f32, tag=f"pm{p}", name=f"pm_{p}_{t}")
                    nc.vector.scalar_tensor_tensor(
                        mt[:], v[:], thr, v[:], op.is_le, op.mult
                    )
                    vn = vp.tile([C, 2 * HW], f32, tag=f"v{p}", name=f"v_{p}_{t}")
                    nc.vector.tensor_tensor(
                        vn[:], mt[:], xt_tiles[p][t + 1][:], op.add
                    )
                    v_cur[p] = vn
    nc.compile()
    return nc


def make_in_maps(x: np.ndarray) -> list[dict]:
    xs = np.ascontiguousarray(x).reshape(B, T, C, HW)
    # prescale x'_t = x_t * 2^t (exact in f32)
    scale = (2.0 ** np.arange(T, dtype=np.float32)).astype(np.float32)
    xs = (xs * scale[None, :, None, None]).astype(np.float32)
    return [
        {
            "x": np.ascontiguousarray(
                xs[i * B_LOC : (i + 1) * B_LOC].transpose(2, 0, 1, 3)
            ).reshape(C, B_LOC, T * HW)
        }
        for i in range(N_CORES)
    ]


def kernel(x: np.ndarray) -> np.ndarray:
    global _nc_cache
    if _nc_cache is None:
        _nc_cache = build_nc()
    res = run_bass_kernel_spmd(_nc_cache, make_in_maps(x), list(range(N_CORES)))
    # out[c, b_loc, t*HW+hw] -> [b, t, c, hw]; spike iff raw == 1
    parts = [
        (res.results[i]["out"].reshape(C, B_LOC, T, HW) == 1).transpose(1, 2, 0, 3)
        for i in range(N_CORES)
    ]
    full = np.concatenate(parts, axis=0)
    return full.reshape(B, T, C, H, W).astype(np.float32)
